# revision 9
# baseline (speedup 1.0000x reference)
"""Contrastive patch loss (InfoNCE over sampled voxel patches) on 8 TRN2 NeuronCores.

Math
----
Every sampled voxel index lives in [0, 512), so cs is a gather of the 512x512
Gram matrix G_b = t2n^T @ t1n: cs[k,l] = G_b[i_k, i_l].  With E_b = exp(G_b/bw)
and c_p[s] = multiplicity of voxel s in patch p:

    loss = -1/(P*B*K) * sum_{b,p,s} c_p[s] *
           log(0.5*diagE_b[s]*(1/CS_b[s,p] + 1/RS_b[s,p]) + eps)

where RS_b = E_b @ C^T and CS_b = E_b^T @ C^T.

Sharding: 8 cores = 2 batches x 4 s-row blocks (m).  Core (b,m) computes only
the m-th 128-row block of RS/CS.  The needed lhsT operands E[m,:]^T and
(E^T)[m,:]^T are produced DIRECTLY as col-block Grams (exp of t1^T@t2n[:,m]
resp. t2^T@t1n[:,m]), so no PE transposes or PSUM->SBUF copies are needed.
The per-core scalar partial sums are returned as (1,1); host adds the 8 and
applies -1/(P*B*K).

SPMD: one program for all cores; the m-dependence enters only through host
data (m-slice feature inputs, m-slice counts).  diag(E)[m-block] is computed
from the elementwise product of the two pre-scaled m-slices (col sums), so no
m-dependent slicing exists on device.

Precision: matmul operands bf16, exp/log/accum f32 (simulated rel err ~5e-5).
"""

import math

import ml_dtypes
import numpy as np

import concourse.bacc as bacc
import concourse.tile as tile
from concourse import hw_specs, mybir
from concourse.bass_utils import run_bass_kernel_spmd

# Pin every ACTIVATE to the one table set that holds ln+exp+copy so the kernel
# pays a single ACT_TABLE_LOAD.
_PIN_SET = "natural_log_exp_and_others"
_orig_get_tables = hw_specs.get_activation_tables


def _pinned_tables(arch):
    tabs = _orig_get_tables(arch)
    return {k: (v if k == _PIN_SET else set()) for k, v in tabs.items()}


bacc.get_activation_tables = _pinned_tables

B, C, S = 2, 256, 512
P, K = 128, 512
BW = 0.05
EPS = 1e-5
N_CORES = 8
F32 = mybir.dt.float32
BF16 = mybir.dt.bfloat16
AX = mybir.AxisListType.X
MUL = mybir.AluOpType.mult
ADD = mybir.AluOpType.add
EXP = mybir.ActivationFunctionType.Exp
LN = mybir.ActivationFunctionType.Ln
COPY = mybir.ActivationFunctionType.Copy


def _build_program():
    nc = bacc.Bacc("TRN2", target_bir_lowering=False, debug=False, num_devices=N_CORES)

    # features packed (q, i*512+s): channel c = 128*i + q
    feat2 = nc.dram_tensor("feat2", [128, 1024], BF16, kind="ExternalInput")
    feat1 = nc.dram_tensor("feat1", [128, 1024], BF16, kind="ExternalInput")
    # m-slices packed (q, i*128+j): s = 128*m + j
    feat2m = nc.dram_tensor("feat2m", [128, 256], BF16, kind="ExternalInput")
    feat1m = nc.dram_tensor("feat1m", [128, 256], BF16, kind="ExternalInput")
    # counts^T blocks (q, a*128+p) = counts[p, 128*a+q]
    cntd = nc.dram_tensor("cnt", [128, 512], BF16, kind="ExternalInput")
    # m-block counts (q, p) = counts[p, 128*m+q]
    cntmd = nc.dram_tensor("cntm", [128, 128], BF16, kind="ExternalInput")
    partial = nc.dram_tensor("partial", [1, 1], F32, kind="ExternalOutput")

    with tile.TileContext(nc) as tc:
        with (
            tc.tile_pool(name="const", bufs=1) as const,
            tc.tile_pool(name="feat", bufs=1) as featp,
            tc.tile_pool(name="sq", bufs=1) as sqp,
            tc.tile_pool(name="e", bufs=1) as ep,
            tc.tile_pool(name="small", bufs=1) as small,
            tc.tile_pool(name="loss", bufs=1) as lossp,
            tc.tile_pool(name="ps_g", bufs=2, space="PSUM") as ps_g,
            tc.tile_pool(name="ps_rc", bufs=1, space="PSUM") as ps_rc,
            tc.tile_pool(name="ps_sm", bufs=1, space="PSUM") as ps_sm,
        ):
            # ---- inputs ----
            f2m = featp.tile([128, 256], BF16, name="f2m", tag="f2m")
            f1m = featp.tile([128, 256], BF16, name="f1m", tag="f1m")
            f2 = featp.tile([128, 1024], BF16, name="f2", tag="f2")
            f1 = featp.tile([128, 1024], BF16, name="f1", tag="f1")
            cntm = featp.tile([128, 128], BF16, name="cntm", tag="cntm")
            cnt = featp.tile([128, 512], BF16, name="cnt", tag="cnt")
            nc.sync.dma_start(out=f2m, in_=feat2m[:, :])
            nc.sync.dma_start(out=f1m, in_=feat1m[:, :])
            nc.sync.dma_start(out=f2, in_=feat2[:, :])
            nc.sync.dma_start(out=f1, in_=feat1[:, :])
            nc.sync.dma_start(out=cntm, in_=cntmd[:, :])
            nc.sync.dma_start(out=cnt, in_=cntd[:, :])

            # ---- constants ----
            ones_col = const.tile([128, 1], BF16, name="ones_col", tag="oc")
            nc.vector.memset(ones_col, 1.0)
            ones_row = const.tile([1, 128], BF16, name="ones_row", tag="orow")
            nc.vector.memset(ones_row, 1.0)
            one_1x1 = const.tile([1, 1], F32, name="one11", tag="one11")
            nc.vector.memset(one_1x1, 1.0)
            eps_col = const.tile([128, 1], F32, name="eps_col", tag="eps")
            nc.vector.memset(eps_col, EPS)
            ln_ibw_col = const.tile([128, 1], F32, name="ln_ibw", tag="libw")
            nc.vector.memset(ln_ibw_col, math.log(1.0 / BW))
            ln_half_col = const.tile([128, 1], F32, name="ln_half", tag="lhalf")
            nc.vector.memset(ln_half_col, math.log(0.5))

            # shared PSUM bank-tiles (sub-sliced; each distinct tile = 1 bank)
            smallrow = ps_sm.tile([1, 512], F32, name="smallrow", tag="smrow")
            smallcol = ps_sm.tile([128, 512], F32, name="smallcol", tag="smcol")

            # ---- m-slice norm path (critical: gates the Gram) ----
            sqm = sqp.tile([128, 512], BF16, name="sqm", tag="sqm")
            sq2m, sq1m = sqm[:, 0:256], sqm[:, 256:512]
            nc.vector.tensor_tensor(out=sq2m, in0=f2m, in1=f2m, op=MUL)
            nc.vector.tensor_tensor(out=sq1m, in0=f1m, in1=f1m, op=MUL)
            ssm_ps = smallrow[0:1, 0:256]
            for i in range(2):
                nc.tensor.matmul(
                    out=ssm_ps[0:1, 0:128], lhsT=ones_col,
                    rhs=sq2m[:, 128 * i : 128 * (i + 1)],
                    start=(i == 0), stop=(i == 1),
                )
            for i in range(2):
                nc.tensor.matmul(
                    out=ssm_ps[0:1, 128:256], lhsT=ones_col,
                    rhs=sq1m[:, 128 * i : 128 * (i + 1)],
                    start=(i == 0), stop=(i == 1),
                )
            lnm = small.tile([1, 256], F32, name="lnm", tag="lnm")
            nc.scalar.activation(out=lnm, in_=ssm_ps, func=LN)
            invm_row = small.tile([1, 256], BF16, name="invm_row", tag="invm")
            nc.scalar.activation(out=invm_row, in_=lnm, func=EXP, scale=-0.5)
            bc_ps = smallcol[:, 0:256]
            nc.tensor.matmul(out=bc_ps, lhsT=ones_row, rhs=invm_row)
            # pre-scaled m-slices (gram rhs): f2ms = t2m * inv2m, f1ms = t1m * inv1m
            f2ms = featp.tile([128, 256], BF16, name="f2ms", tag="f2ms")
            f1ms = featp.tile([128, 256], BF16, name="f1ms", tag="f1ms")
            for i in range(2):
                isl = slice(128 * i, 128 * (i + 1))
                nc.vector.tensor_tensor(
                    out=f2ms[:, isl], in0=f2m[:, isl], in1=bc_ps[:, 0:128], op=MUL
                )
                nc.vector.tensor_tensor(
                    out=f1ms[:, isl], in0=f1m[:, isl], in1=bc_ps[:, 128:256], op=MUL
                )

            # ---- full-range sumsq -> inv/BW column forms (gates the exps) ----
            sq2 = sqp.tile([128, 1024], BF16, name="sq2", tag="sq2")
            sq1 = sqp.tile([128, 1024], BF16, name="sq1", tag="sq1")
            lns = small.tile([1, 1024], F32, name="lns", tag="lns")
            ln2, ln1 = lns[0:1, 0:512], lns[0:1, 512:1024]
            ss_ps = []
            for nm, f, sq in (("2", f2, sq2), ("1", f1, sq1)):
                nc.vector.tensor_tensor(out=sq[:, 0:512], in0=f[:, 0:512], in1=f[:, 0:512], op=MUL)
                nc.vector.tensor_tensor(out=sq[:, 512:1024], in0=f[:, 512:1024], in1=f[:, 512:1024], op=MUL)
                ssp = ps_sm.tile([1, 512], F32, name=f"ss{nm}_ps", tag=f"ss{nm}")
                for i in range(2):
                    nc.tensor.matmul(
                        out=ssp, lhsT=ones_col, rhs=sq[:, 512 * i : 512 * (i + 1)],
                        start=(i == 0), stop=(i == 1),
                    )
                ss_ps.append(ssp)
            nc.scalar.activation(out=ln2, in_=ss_ps[0], func=LN)
            nc.scalar.activation(out=ln1, in_=ss_ps[1], func=LN)
            # transpose (1,512) rows into (128,4) cols, then exp(-0.5*ln + ln(1/BW))
            invbw = small.tile([128, 8], F32, name="invbw", tag="invbw")
            tr_ps = smallcol[:, 256:264]
            for a in range(4):
                nc.tensor.transpose(
                    out=tr_ps[:, a : a + 1],
                    in_=ln2[0:1, 128 * a : 128 * (a + 1)], identity=one_1x1,
                )
            for a in range(4):
                nc.tensor.transpose(
                    out=tr_ps[:, 4 + a : 5 + a],
                    in_=ln1[0:1, 128 * a : 128 * (a + 1)], identity=one_1x1,
                )
            nc.scalar.activation(
                out=invbw, in_=tr_ps, func=EXP, scale=-0.5, bias=ln_ibw_col
            )
            inv2bw, inv1bw = invbw[:, 0:4], invbw[:, 4:8]

            # ---- diag path: cs_diag = colsum(f2ms * f1ms); half_dcol = 0.5*exp(cs/BW)
            dprod = sqp.tile([128, 256], BF16, name="dprod", tag="dprod")
            nc.vector.tensor_tensor(out=dprod, in0=f2ms, in1=f1ms, op=MUL)
            dps = smallrow[0:1, 256:384]
            for i in range(2):
                nc.tensor.matmul(
                    out=dps, lhsT=ones_col, rhs=dprod[:, 128 * i : 128 * (i + 1)],
                    start=(i == 0), stop=(i == 1),
                )
            drow = small.tile([1, 128], F32, name="drow", tag="drow")
            nc.scalar.activation(out=drow, in_=dps, func=COPY)
            dcol_ps = smallcol[:, 264:265]
            nc.tensor.transpose(out=dcol_ps, in_=drow, identity=one_1x1)
            half_dcol = small.tile([128, 1], F32, name="half_dcol", tag="hdc")
            nc.scalar.activation(
                out=half_dcol, in_=dcol_ps, func=EXP, scale=1.0 / BW, bias=ln_half_col
            )

            # ---- Grams: ET = (E^T)[m,:]^T blocks, EM = E[m,:]^T blocks ----
            et = ep.tile([128, 512], BF16, name="et", tag="et")
            em = ep.tile([128, 512], BF16, name="em", tag="em")
            for dst, lhs_full, rhs_m, sc in (
                (et, f2, f1ms, inv2bw),
                (em, f1, f2ms, inv1bw),
            ):
                for a in range(4):
                    g_ps = ps_g.tile([128, 128], F32, name="g_ps", tag="g_ps")
                    for i in range(2):
                        nc.tensor.matmul(
                            out=g_ps,
                            lhsT=lhs_full[:, 512 * i + 128 * a : 512 * i + 128 * (a + 1)],
                            rhs=rhs_m[:, 128 * i : 128 * (i + 1)],
                            start=(i == 0), stop=(i == 1),
                        )
                    nc.scalar.activation(
                        out=dst[:, 128 * a : 128 * (a + 1)], in_=g_ps,
                        func=EXP, scale=sc[:, a : a + 1],
                    )

            # ---- RS/CS and loss ----
            rc_ps = ps_rc.tile([128, 256], F32, name="rc_ps", tag="rc_ps")
            cs_ps, rs_ps = rc_ps[:, 0:128], rc_ps[:, 128:256]
            for a in range(4):
                asl = slice(128 * a, 128 * (a + 1))
                nc.tensor.matmul(
                    out=cs_ps, lhsT=et[:, asl], rhs=cnt[:, asl],
                    start=(a == 0), stop=(a == 3),
                )
            for a in range(4):
                asl = slice(128 * a, 128 * (a + 1))
                nc.tensor.matmul(
                    out=rs_ps, lhsT=em[:, asl], rhs=cnt[:, asl],
                    start=(a == 0), stop=(a == 3),
                )
            cinv = lossp.tile([128, 128], F32, name="cinv", tag="cinv")
            rinv = lossp.tile([128, 128], F32, name="rinv", tag="rinv")
            nc.vector.reciprocal(out=cinv, in_=cs_ps)
            nc.vector.reciprocal(out=rinv, in_=rs_ps)
            ssum = lossp.tile([128, 128], F32, name="ssum", tag="ssum")
            nc.vector.tensor_tensor(out=ssum, in0=rinv, in1=cinv, op=ADD)
            g = lossp.tile([128, 128], F32, name="g", tag="g")
            nc.scalar.activation(
                out=g, in_=ssum, func=LN, scale=half_dcol, bias=eps_col
            )
            scr = lossp.tile([128, 128], BF16, name="scr", tag="scr")
            nc.vector.tensor_tensor(out=scr, in0=g, in1=cntm, op=MUL)
            tot_ps = smallrow[0:1, 384:512]
            nc.tensor.matmul(out=tot_ps, lhsT=ones_col, rhs=scr)
            tot = small.tile([1, 1], F32, name="tot", tag="totsb")
            nc.vector.tensor_reduce(out=tot, in_=tot_ps, axis=AX, op=ADD)
            nc.sync.dma_start(out=partial[:, :], in_=tot)

    nc.compile()
    return nc


_NC = None


def _run(t2_feat, t1_feat, idx, trace=False, trace_kwargs=None):
    global _NC
    if _NC is None:
        _NC = _build_program()

    t2 = np.asarray(t2_feat, np.float32).reshape(B, C, S)
    t1 = np.asarray(t1_feat, np.float32).reshape(B, C, S)
    idx = np.asarray(idx)

    counts = np.zeros((P, S), np.float32)
    np.add.at(counts, (np.arange(P)[:, None], idx), 1.0)
    cnt_dev = np.ascontiguousarray(
        counts.T.reshape(4, 128, 128).transpose(1, 0, 2).reshape(128, 512)
    ).astype(ml_dtypes.bfloat16)

    packed = {}
    for nm, t in (("2", t2), ("1", t1)):
        packed[nm] = [
            np.ascontiguousarray(
                t[b].reshape(2, 128, 512).transpose(1, 0, 2).reshape(128, 1024)
            ).astype(ml_dtypes.bfloat16)
            for b in range(B)
        ]

    in_maps = []
    for core in range(N_CORES):
        b, m = divmod(core, 4)
        p2, p1 = packed["2"][b], packed["1"][b]
        msl = [slice(512 * i + 128 * m, 512 * i + 128 * (m + 1)) for i in range(2)]
        in_maps.append(
            {
                "feat2": p2,
                "feat1": p1,
                "feat2m": np.ascontiguousarray(
                    np.concatenate([p2[:, s] for s in msl], axis=1)
                ),
                "feat1m": np.ascontiguousarray(
                    np.concatenate([p1[:, s] for s in msl], axis=1)
                ),
                "cnt": cnt_dev,
                "cntm": np.ascontiguousarray(
                    counts[:, 128 * m : 128 * (m + 1)].T
                ).astype(ml_dtypes.bfloat16),
            }
        )

    kwargs = {}
    if trace:
        kwargs = dict(trace=True, trace_kwargs=trace_kwargs or {})
    res = run_bass_kernel_spmd(_NC, in_maps, core_ids=list(range(N_CORES)), **kwargs)
    total = sum(float(r["partial"][0, 0]) for r in res.results)
    loss = -total / (P * B * K)
    return np.array(loss, dtype=np.float32), res


def kernel(t2_feat, t1_feat, idx):
    out, _ = _run(t2_feat, t1_feat, idx)
    return out


# revision 10
# speedup vs baseline: 1.2539x; 1.2539x over previous
"""Contrastive patch loss (InfoNCE over sampled voxel patches) on 8 TRN2 NeuronCores.

Math
----
Every sampled voxel index lives in [0, 512), so cs is a gather of the 512x512
Gram matrix G_b = t2n^T @ t1n: cs[k,l] = G_b[i_k, i_l].  With E_b = exp(G_b/bw)
and c_p[s] = multiplicity of voxel s in patch p:

    loss = -1/(P*B*K) * sum_{b,p,s} c_p[s] *
           log(0.5*diagE_b[s]*(1/CS_b[s,p] + 1/RS_b[s,p]) + eps)

where RS_b = E_b @ C^T and CS_b = E_b^T @ C^T.

Sharding: 8 cores = 2 batches x 4 s-row blocks (m).  Core (b,m) computes only
the m-th 128-row block of RS/CS.  The needed lhsT operands E[m,:]^T and
(E^T)[m,:]^T are produced DIRECTLY as col-block Grams (exp of t1^T@t2n[:,m]
resp. t2^T@t1n[:,m]); no PE transposes or PSUM->SBUF copies.  One SPMD
program: all m-dependence enters via host-prepared inputs (m-slice features,
m-slice counts); diag(E)[m-block] comes from the elementwise product of the
two pre-scaled m-slices.  Per-core (1,1) partials; host sums and scales.

Precision: features fp8e4 (DMA 2x smaller, PE 2x faster; verified ~4e-4 rel
err in simulation), counts + E in bf16, exp/log/accumulation f32.
"""

import math

import ml_dtypes
import numpy as np

import concourse.bacc as bacc
import concourse.tile as tile
from concourse import hw_specs, mybir
from concourse.bass_utils import run_bass_kernel_spmd

# Pin every ACTIVATE to the one table set that holds ln+exp+copy so the kernel
# pays a single ACT_TABLE_LOAD.
_PIN_SET = "natural_log_exp_and_others"
_orig_get_tables = hw_specs.get_activation_tables


def _pinned_tables(arch):
    tabs = _orig_get_tables(arch)
    return {k: (v if k == _PIN_SET else set()) for k, v in tabs.items()}


bacc.get_activation_tables = _pinned_tables

B, C, S = 2, 256, 512
P, K = 128, 512
BW = 0.05
EPS = 1e-5
N_CORES = 8
F32 = mybir.dt.float32
BF16 = mybir.dt.bfloat16
FP8 = mybir.dt.float8e4
AX = mybir.AxisListType.X
MUL = mybir.AluOpType.mult
ADD = mybir.AluOpType.add
EXP = mybir.ActivationFunctionType.Exp
LN = mybir.ActivationFunctionType.Ln

USE_FP8 = True   # fp8e4 feature tensors + fp8 grams
USE_DR = True    # DoubleRow perf mode for the grams (fp8 only)
FDT = FP8 if USE_FP8 else BF16
NPDT = ml_dtypes.float8_e4m3 if USE_FP8 else ml_dtypes.bfloat16


def _build_program():
    nc = bacc.Bacc("TRN2", target_bir_lowering=False, debug=False, num_devices=N_CORES)

    # features packed (q, i*512+s): channel c = 128*i + q
    feat2 = nc.dram_tensor("feat2", [128, 1024], FDT, kind="ExternalInput")
    feat1 = nc.dram_tensor("feat1", [128, 1024], FDT, kind="ExternalInput")
    # m-slices packed (q, i*128+j): s = 128*m + j
    feat2m = nc.dram_tensor("feat2m", [128, 256], FDT, kind="ExternalInput")
    feat1m = nc.dram_tensor("feat1m", [128, 256], FDT, kind="ExternalInput")
    # counts^T blocks (q, a*128+p) = counts[p, 128*a+q]
    cntd = nc.dram_tensor("cnt", [128, 512], BF16, kind="ExternalInput")
    # m-block counts (q, p) = counts[p, 128*m+q]
    cntmd = nc.dram_tensor("cntm", [128, 128], BF16, kind="ExternalInput")
    partial = nc.dram_tensor("partial", [1, 1], F32, kind="ExternalOutput")

    with tile.TileContext(nc) as tc:
        with (
            tc.tile_pool(name="const", bufs=1) as const,
            tc.tile_pool(name="feat", bufs=1) as featp,
            tc.tile_pool(name="sq", bufs=1) as sqp,
            tc.tile_pool(name="e", bufs=1) as ep,
            tc.tile_pool(name="small", bufs=1) as small,
            tc.tile_pool(name="loss", bufs=1) as lossp,
            tc.tile_pool(name="ps_g", bufs=2, space="PSUM") as ps_g,
            tc.tile_pool(name="ps_rc", bufs=1, space="PSUM") as ps_rc,
            tc.tile_pool(name="ps_sm", bufs=1, space="PSUM") as ps_sm,
        ):
            # ---- inputs (m-slices first: they gate the critical chain) ----
            f2m = featp.tile([128, 256], FDT, name="f2m", tag="f2m")
            f1m = featp.tile([128, 256], FDT, name="f1m", tag="f1m")
            f2 = featp.tile([128, 1024], FDT, name="f2", tag="f2")
            f1 = featp.tile([128, 1024], FDT, name="f1", tag="f1")
            cntm = featp.tile([128, 128], BF16, name="cntm", tag="cntm")
            cnt = featp.tile([128, 512], BF16, name="cnt", tag="cnt")
            nc.sync.dma_start(out=f2m, in_=feat2m[:, :])
            nc.sync.dma_start(out=f1m, in_=feat1m[:, :])
            nc.sync.dma_start(out=f2, in_=feat2[:, :])
            nc.sync.dma_start(out=f1, in_=feat1[:, :])
            nc.sync.dma_start(out=cntm, in_=cntmd[:, :])
            nc.sync.dma_start(out=cnt, in_=cntd[:, :])

            # ---- constants ----
            ones_col = const.tile([128, 1], BF16, name="ones_col", tag="oc")
            nc.vector.memset(ones_col, 1.0)
            ones_row = const.tile([1, 128], BF16, name="ones_row", tag="orow")
            nc.vector.memset(ones_row, 1.0)
            one_1x1 = const.tile([1, 1], F32, name="one11", tag="one11")
            nc.vector.memset(one_1x1, 1.0)
            eps_col = const.tile([128, 1], F32, name="eps_col", tag="eps")
            nc.vector.memset(eps_col, EPS)
            ln_ibw_col = const.tile([128, 1], F32, name="ln_ibw", tag="libw")
            nc.vector.memset(ln_ibw_col, math.log(1.0 / BW))
            ln_half_col = const.tile([128, 1], F32, name="ln_half", tag="lhalf")
            nc.vector.memset(ln_half_col, math.log(0.5))

            # shared PSUM bank-tiles (sub-sliced; each distinct tile = 1 bank)
            smallrow = ps_sm.tile([1, 512], F32, name="smallrow", tag="smrow")
            smallcol = ps_sm.tile([128, 512], F32, name="smallcol", tag="smcol")

            # ==== critical m-slice norm path (floor 0) ====
            sqm = sqp.tile([128, 512], BF16, name="sqm", tag="sqm")
            sq2m, sq1m = sqm[:, 0:256], sqm[:, 256:512]
            nc.vector.tensor_tensor(out=sq2m, in0=f2m, in1=f2m, op=MUL)
            nc.vector.tensor_tensor(out=sq1m, in0=f1m, in1=f1m, op=MUL)
            ssm_ps = smallrow[0:1, 0:256]
            for i in range(2):
                nc.tensor.matmul(
                    out=ssm_ps[0:1, 0:128], lhsT=ones_col,
                    rhs=sq2m[:, 128 * i : 128 * (i + 1)],
                    start=(i == 0), stop=(i == 1),
                )
            for i in range(2):
                nc.tensor.matmul(
                    out=ssm_ps[0:1, 128:256], lhsT=ones_col,
                    rhs=sq1m[:, 128 * i : 128 * (i + 1)],
                    start=(i == 0), stop=(i == 1),
                )
            lnm = small.tile([1, 256], F32, name="lnm", tag="lnm")
            nc.scalar.activation(out=lnm, in_=ssm_ps, func=LN)
            invm_row = small.tile([1, 256], BF16, name="invm_row", tag="invm")
            nc.scalar.activation(out=invm_row, in_=lnm, func=EXP, scale=-0.5)
            bc_ps = smallcol[:, 0:256]
            nc.tensor.matmul(out=bc_ps, lhsT=ones_row, rhs=invm_row)
            # pre-scaled m-slices (gram rhs): f2ms = t2m * inv2m, f1ms = t1m * inv1m
            f2ms = featp.tile([128, 256], FDT, name="f2ms", tag="f2ms")
            f1ms = featp.tile([128, 256], FDT, name="f1ms", tag="f1ms")
            for i in range(2):
                isl = slice(128 * i, 128 * (i + 1))
                nc.vector.tensor_tensor(
                    out=f2ms[:, isl], in0=f2m[:, isl], in1=bc_ps[:, 0:128], op=MUL
                )
                nc.vector.tensor_tensor(
                    out=f1ms[:, isl], in0=f1m[:, isl], in1=bc_ps[:, 128:256], op=MUL
                )

            # ==== full-range sumsq -> inv/BW col forms (floor 1: after m-path) ====
            with tc.tile_wait_until(0.0004):
                sq2 = sqp.tile([128, 1024], BF16, name="sq2", tag="sq2")
                sq1 = sqp.tile([128, 1024], BF16, name="sq1", tag="sq1")
                lns = small.tile([1, 1024], F32, name="lns", tag="lns")
                ln2, ln1 = lns[0:1, 0:512], lns[0:1, 512:1024]
                ss_ps = []
                for nm, f, sq in (("2", f2, sq2), ("1", f1, sq1)):
                    nc.gpsimd.tensor_tensor(
                        out=sq[:, 0:512], in0=f[:, 0:512], in1=f[:, 0:512], op=MUL
                    )
                    nc.gpsimd.tensor_tensor(
                        out=sq[:, 512:1024], in0=f[:, 512:1024], in1=f[:, 512:1024], op=MUL
                    )
                    ssp = ps_sm.tile([1, 512], F32, name=f"ss{nm}_ps", tag=f"ss{nm}")
                    for i in range(2):
                        nc.tensor.matmul(
                            out=ssp, lhsT=ones_col, rhs=sq[:, 512 * i : 512 * (i + 1)],
                            start=(i == 0), stop=(i == 1),
                        )
                    ss_ps.append(ssp)
                nc.scalar.activation(out=ln2, in_=ss_ps[0], func=LN)
                nc.scalar.activation(out=ln1, in_=ss_ps[1], func=LN)
                # transpose (1,512) rows to (128,4) cols; exp(-0.5*ln + ln(1/BW))
                invbw = small.tile([128, 8], F32, name="invbw", tag="invbw")
                tr_ps = smallcol[:, 256:264]
                for a in range(4):
                    nc.tensor.transpose(
                        out=tr_ps[:, a : a + 1],
                        in_=ln2[0:1, 128 * a : 128 * (a + 1)], identity=one_1x1,
                    )
                for a in range(4):
                    nc.tensor.transpose(
                        out=tr_ps[:, 4 + a : 5 + a],
                        in_=ln1[0:1, 128 * a : 128 * (a + 1)], identity=one_1x1,
                    )
                nc.scalar.activation(
                    out=invbw, in_=tr_ps, func=EXP, scale=-0.5, bias=ln_ibw_col
                )
                inv2bw, inv1bw = invbw[:, 0:4], invbw[:, 4:8]

            # ==== Grams (floor 2) ====
            with tc.tile_wait_until(0.0008):
                et = ep.tile([128, 512], BF16, name="et", tag="et")
                em = ep.tile([128, 512], BF16, name="em", tag="em")
                if USE_DR:
                    f2r = f2.rearrange("q (i s) -> q i s", i=2)
                    f1r = f1.rearrange("q (i s) -> q i s", i=2)
                    f2msr = f2ms.rearrange("q (i j) -> q i j", i=2)
                    f1msr = f1ms.rearrange("q (i j) -> q i j", i=2)
                    for dst, lhs_r, rhs_r, sc in (
                        (et, f2r, f1msr, inv2bw),
                        (em, f1r, f2msr, inv1bw),
                    ):
                        for a in range(4):
                            g_ps = ps_g.tile([128, 128], F32, name="g_ps", tag="g_ps")
                            nc.tensor.matmul(
                                out=g_ps,
                                lhsT=lhs_r[:, :, 128 * a : 128 * (a + 1)],
                                rhs=rhs_r,
                                perf_mode=mybir.MatmulPerfMode.DoubleRow,
                            )
                            nc.scalar.activation(
                                out=dst[:, 128 * a : 128 * (a + 1)], in_=g_ps,
                                func=EXP, scale=sc[:, a : a + 1],
                            )
                else:
                    for dst, lhs_full, rhs_m, sc in (
                        (et, f2, f1ms, inv2bw),
                        (em, f1, f2ms, inv1bw),
                    ):
                        for a in range(4):
                            g_ps = ps_g.tile([128, 128], F32, name="g_ps", tag="g_ps")
                            for i in range(2):
                                nc.tensor.matmul(
                                    out=g_ps,
                                    lhsT=lhs_full[:, 512 * i + 128 * a : 512 * i + 128 * (a + 1)],
                                    rhs=rhs_m[:, 128 * i : 128 * (i + 1)],
                                    start=(i == 0), stop=(i == 1),
                                )
                            nc.scalar.activation(
                                out=dst[:, 128 * a : 128 * (a + 1)], in_=g_ps,
                                func=EXP, scale=sc[:, a : a + 1],
                            )

            # ==== diag path (floor 3; only needed by the final Ln) ====
            with tc.tile_wait_until(0.0012):
                dprod = sqp.tile([128, 256], BF16, name="dprod", tag="dprod")
                nc.gpsimd.tensor_tensor(out=dprod, in0=f2ms, in1=f1ms, op=MUL)
                dps = smallrow[0:1, 256:384]
                for i in range(2):
                    nc.tensor.matmul(
                        out=dps, lhsT=ones_col, rhs=dprod[:, 128 * i : 128 * (i + 1)],
                        start=(i == 0), stop=(i == 1),
                    )
                drow = small.tile([1, 128], F32, name="drow", tag="drow")
                nc.vector.tensor_copy(out=drow, in_=dps)
                dcol_ps = smallcol[:, 264:265]
                nc.tensor.transpose(out=dcol_ps, in_=drow, identity=one_1x1)
                half_dcol = small.tile([128, 1], F32, name="half_dcol", tag="hdc")
                nc.scalar.activation(
                    out=half_dcol, in_=dcol_ps, func=EXP, scale=1.0 / BW,
                    bias=ln_half_col,
                )

            # ==== RS/CS and loss (floor 4) ====
            with tc.tile_wait_until(0.0016):
                rc_ps = ps_rc.tile([128, 256], F32, name="rc_ps", tag="rc_ps")
                cs_ps, rs_ps = rc_ps[:, 0:128], rc_ps[:, 128:256]
                for a in range(4):
                    asl = slice(128 * a, 128 * (a + 1))
                    nc.tensor.matmul(
                        out=cs_ps, lhsT=et[:, asl], rhs=cnt[:, asl],
                        start=(a == 0), stop=(a == 3),
                    )
                for a in range(4):
                    asl = slice(128 * a, 128 * (a + 1))
                    nc.tensor.matmul(
                        out=rs_ps, lhsT=em[:, asl], rhs=cnt[:, asl],
                        start=(a == 0), stop=(a == 3),
                    )
                cinv = lossp.tile([128, 128], F32, name="cinv", tag="cinv")
                rinv = lossp.tile([128, 128], F32, name="rinv", tag="rinv")
                nc.vector.reciprocal_approx_fast(out=cinv, in_=cs_ps)
                nc.vector.reciprocal_approx_fast(out=rinv, in_=rs_ps)
                ssum = lossp.tile([128, 128], F32, name="ssum", tag="ssum")
                nc.vector.tensor_tensor(out=ssum, in0=rinv, in1=cinv, op=ADD)
                g = lossp.tile([128, 128], F32, name="g", tag="g")
                nc.scalar.activation(
                    out=g, in_=ssum, func=LN, scale=half_dcol, bias=eps_col
                )
                scr = lossp.tile([128, 128], BF16, name="scr", tag="scr")
                nc.vector.tensor_tensor(out=scr, in0=g, in1=cntm, op=MUL)
                tot_ps = smallrow[0:1, 384:512]
                nc.tensor.matmul(out=tot_ps, lhsT=ones_col, rhs=scr)
                tot = small.tile([1, 1], F32, name="tot", tag="totsb")
                nc.vector.tensor_reduce(out=tot, in_=tot_ps, axis=AX, op=ADD)
                nc.sync.dma_start(out=partial[:, :], in_=tot)

    nc.compile()
    return nc


_NC = None


def _run(t2_feat, t1_feat, idx, trace=False, trace_kwargs=None):
    global _NC
    if _NC is None:
        _NC = _build_program()

    t2 = np.asarray(t2_feat, np.float32).reshape(B, C, S)
    t1 = np.asarray(t1_feat, np.float32).reshape(B, C, S)
    idx = np.asarray(idx)

    counts = np.zeros((P, S), np.float32)
    np.add.at(counts, (np.arange(P)[:, None], idx), 1.0)
    cnt_dev = np.ascontiguousarray(
        counts.T.reshape(4, 128, 128).transpose(1, 0, 2).reshape(128, 512)
    ).astype(ml_dtypes.bfloat16)

    packed = {}
    for nm, t in (("2", t2), ("1", t1)):
        packed[nm] = [
            np.ascontiguousarray(
                t[b].reshape(2, 128, 512).transpose(1, 0, 2).reshape(128, 1024)
            ).astype(NPDT)
            for b in range(B)
        ]

    in_maps = []
    for core in range(N_CORES):
        b, m = divmod(core, 4)
        p2, p1 = packed["2"][b], packed["1"][b]
        msl = [slice(512 * i + 128 * m, 512 * i + 128 * (m + 1)) for i in range(2)]
        in_maps.append(
            {
                "feat2": p2,
                "feat1": p1,
                "feat2m": np.ascontiguousarray(
                    np.concatenate([p2[:, s] for s in msl], axis=1)
                ),
                "feat1m": np.ascontiguousarray(
                    np.concatenate([p1[:, s] for s in msl], axis=1)
                ),
                "cnt": cnt_dev,
                "cntm": np.ascontiguousarray(
                    counts[:, 128 * m : 128 * (m + 1)].T
                ).astype(ml_dtypes.bfloat16),
            }
        )

    kwargs = {}
    if trace:
        kwargs = dict(trace=True, trace_kwargs=trace_kwargs or {})
    res = run_bass_kernel_spmd(_NC, in_maps, core_ids=list(range(N_CORES)), **kwargs)
    total = sum(float(r["partial"][0, 0]) for r in res.results)
    loss = -total / (P * B * K)
    return np.array(loss, dtype=np.float32), res


def kernel(t2_feat, t1_feat, idx):
    out, _ = _run(t2_feat, t1_feat, idx)
    return out
